# revision 6
# baseline (speedup 1.0000x reference)
"""Multi-head cross attention (B=32, Nq=16384, Nk=31, d_model=64, H=4) on 8 trn2 cores.

Strategy: pure data parallel over batch (4 batches per core). Per batch b the
whole attention is restructured so the only large tensor (Q) is streamed once:

  scores^T[k,q] = (Kblk/sqrt(dh)).T @ Q^T      Kblk: [64, 124] block-diag per head
  E = exp(scores^T + mask_bias)                mask_bias: 0 / -1e30 per k-row
  R = M124.T @ E                               M124: [124,124] block-ones -> per-head row sums
  En = E / R                                   softmax weights (transposed layout)
  out[q,:] = En.T @ VW + b_out                 VW[31h:31h+31,:] = V_h @ W_out[:,hblk].T

Q^T tiles are produced on-chip with PE transposes of naturally-loaded Q tiles.
"""

import os
import sys

for _p in ("/opt/trn_rl_repo", "/opt/pypackages",
           "/root/.axon_site/_ro/trn_rl_repo", "/root/.axon_site/_ro/pypackages"):
    if os.path.isdir(_p) and _p not in sys.path:
        sys.path.insert(0, _p)

import math
import numpy as np

import concourse.bass as bass
import concourse.tile as tile
from concourse import bacc, mybir
from concourse.bass_utils import run_bass_kernel_spmd
from concourse.masks import make_identity

B, NQ, NK, D = 32, 16384, 31, 64
H, DH = 4, 16
SCALE = math.sqrt(DH)
NCORES = 8
BL = B // NCORES          # batches per core
TQ = 512                  # queries per inner iteration
NT = NQ // TQ             # iterations per batch
KB = H * NK               # 124 stacked key rows

MASK_NEG = -1e30

_PROG_CACHE: dict = {}


def _build(mode: str = "f32r", nq: int = NQ):
    """Build the per-core Bass program. mode: 'f32' | 'f32r' for matmul inputs."""
    f32 = mybir.dt.float32
    mmdt = mybir.dt.float32r if mode == "f32r" else f32

    def mmcast(ap):
        return ap

    nt = nq // TQ
    nc = bacc.Bacc("TRN2", target_bir_lowering=False, debug=False, num_devices=NCORES)
    q = nc.dram_tensor("q", [BL * nq, D], f32, kind="ExternalInput").ap()
    kblk = nc.dram_tensor("kblk", [BL, D, KB], mmdt, kind="ExternalInput").ap()
    vw = nc.dram_tensor("vw", [BL, KB, D], mmdt, kind="ExternalInput").ap()
    mbias = nc.dram_tensor("mbias", [BL, KB, 1], f32, kind="ExternalInput").ap()
    m124 = nc.dram_tensor("m124", [KB, KB], mmdt, kind="ExternalInput").ap()
    bias_bc = nc.dram_tensor("bias_bc", [128, (TQ // 128) * D], f32,
                             kind="ExternalInput").ap()
    o = nc.dram_tensor("o", [BL * nq, D], f32, kind="ExternalOutput").ap()

    ntile = TQ // 128  # q-subtiles of 128 per iteration

    with tile.TileContext(nc) as tc:
        with (
            tc.tile_pool(name="singles", bufs=1) as singles,
            tc.tile_pool(name="qin", bufs=3) as qin_pool,
            tc.tile_pool(name="qt_ps", bufs=2, space="PSUM") as qtps_pool,
            tc.tile_pool(name="qt_sb", bufs=2) as qtsb_pool,
            tc.tile_pool(name="st", bufs=2, space="PSUM") as st_pool,
            tc.tile_pool(name="et", bufs=2) as et_pool,
            tc.tile_pool(name="r", bufs=2, space="PSUM") as r_pool,
            tc.tile_pool(name="rinv", bufs=2) as rinv_pool,
            tc.tile_pool(name="en", bufs=2) as en_pool,
            tc.tile_pool(name="u", bufs=2, space="PSUM") as u_pool,
            tc.tile_pool(name="osb", bufs=3) as o_pool,
        ):
            ident = singles.tile([128, 128], f32)
            make_identity(nc, ident)
            kblk_sb = singles.tile([D, BL, KB], mmdt)
            vw_sb = singles.tile([KB, BL, D], mmdt)
            mb_sb = singles.tile([KB, BL], f32)
            m124_sb = singles.tile([KB, KB], mmdt)
            bias_sb = singles.tile([128, ntile * D], f32)
            for b in range(BL):
                nc.sync.dma_start(out=kblk_sb[:, b, :], in_=kblk[b])
                nc.sync.dma_start(out=vw_sb[:, b, :], in_=vw[b])
                nc.sync.dma_start(out=mb_sb[:, b : b + 1], in_=mbias[b])
            nc.sync.dma_start(out=m124_sb, in_=m124)
            nc.sync.dma_start(out=bias_sb, in_=bias_bc)

            for b in range(BL):
                for i in range(nt):
                    row0 = b * nq + i * TQ
                    qin = qin_pool.tile([128, ntile, D], f32)
                    nc.sync.dma_start(
                        out=qin,
                        in_=q[row0 : row0 + TQ].rearrange("(t p) d -> p t d", p=128),
                    )
                    qt_ps = qtps_pool.tile([D, ntile, 128], f32)
                    for t in range(ntile):
                        nc.tensor.transpose(qt_ps[:, t, :], qin[:, t, :], ident)
                    qt_sb = qtsb_pool.tile([D, ntile, 128], mmdt)
                    nc.scalar.copy(qt_sb, qt_ps)

                    st = st_pool.tile([KB, TQ], f32)
                    nc.tensor.matmul(
                        st,
                        mmcast(kblk_sb[:, b, :]),
                        mmcast(qt_sb.rearrange("d t p -> d (t p)")),
                        start=True,
                        stop=True,
                    )
                    et = et_pool.tile([KB, TQ], mmdt)
                    nc.scalar.activation(
                        et, st, mybir.ActivationFunctionType.Exp,
                        bias=mb_sb[:, b : b + 1], scale=1.0,
                    )
                    r = r_pool.tile([KB, TQ], f32)
                    nc.tensor.matmul(r, mmcast(m124_sb), mmcast(et), start=True, stop=True)
                    rinv = rinv_pool.tile([KB, TQ], f32)
                    nc.vector.reciprocal(rinv, r)
                    en = en_pool.tile([KB, TQ], mmdt)
                    nc.vector.tensor_mul(en, et, rinv)

                    u = u_pool.tile([128, ntile, D], f32)
                    for t in range(ntile):
                        nc.tensor.matmul(
                            u[:, t, :],
                            mmcast(en[:, t * 128 : (t + 1) * 128]),
                            mmcast(vw_sb[:, b, :]),
                            start=True,
                            stop=True,
                        )
                    osb = o_pool.tile([128, ntile, D], f32)
                    nc.vector.tensor_add(
                        osb.rearrange("p t d -> p (t d)"),
                        u.rearrange("p t d -> p (t d)"),
                        bias_sb,
                    )
                    nc.sync.dma_start(
                        out=o[row0 : row0 + TQ].rearrange("(t p) d -> p t d", p=128),
                        in_=osb,
                    )
    nc.compile()
    return nc


def _build_bf16(nq: int = NQ):
    """bf16 path: DMA-transposed Q loads, bf16 matmuls, approx-free softmax
    normalization via a tiny per-head row-sum matmul + reciprocal + expand."""
    f32 = mybir.dt.float32
    bf16 = mybir.dt.bfloat16
    nt = nq // TQ
    ntile = TQ // 128

    nc = bacc.Bacc("TRN2", target_bir_lowering=False, debug=False, num_devices=NCORES)
    q = nc.dram_tensor("q", [BL * nq, D], bf16, kind="ExternalInput").ap()
    kblk = nc.dram_tensor("kblk", [BL, D, KB], bf16, kind="ExternalInput").ap()
    vw = nc.dram_tensor("vw", [BL, KB, D], bf16, kind="ExternalInput").ap()
    mbias = nc.dram_tensor("mbias", [BL, KB, 1], f32, kind="ExternalInput").ap()
    onesb = nc.dram_tensor("onesb", [KB, H], bf16, kind="ExternalInput").ap()
    exp4 = nc.dram_tensor("exp4", [H, KB], bf16, kind="ExternalInput").ap()
    bias_bc = nc.dram_tensor("bias_bc", [128, ntile * D], f32,
                             kind="ExternalInput").ap()
    o = nc.dram_tensor("o", [BL * nq, D], f32, kind="ExternalOutput").ap()

    with tile.TileContext(nc) as tc:
        with (
            tc.tile_pool(name="singles", bufs=1) as singles,
            tc.tile_pool(name="qt", bufs=3) as qt_pool,
            tc.tile_pool(name="st", bufs=2, space="PSUM") as st_pool,
            tc.tile_pool(name="et", bufs=2) as et_pool,
            tc.tile_pool(name="r4", bufs=2, space="PSUM") as r4_pool,
            tc.tile_pool(name="rinv", bufs=2) as rinv_pool,
            tc.tile_pool(name="rx", bufs=2, space="PSUM") as rx_pool,
            tc.tile_pool(name="en", bufs=2) as en_pool,
            tc.tile_pool(name="u", bufs=2, space="PSUM") as u_pool,
            tc.tile_pool(name="osb", bufs=3) as o_pool,
        ):
            kblk_sb = singles.tile([D, BL, KB], bf16)
            vw_sb = singles.tile([KB, BL, D], bf16)
            mb_sb = singles.tile([KB, BL], f32)
            onesb_sb = singles.tile([KB, H], bf16)
            exp4_sb = singles.tile([H, KB], bf16)
            bias_sb = singles.tile([128, ntile * D], f32)
            for b in range(BL):
                nc.sync.dma_start(out=kblk_sb[:, b, :], in_=kblk[b])
                nc.sync.dma_start(out=vw_sb[:, b, :], in_=vw[b])
                nc.sync.dma_start(out=mb_sb[:, b : b + 1], in_=mbias[b])
            nc.sync.dma_start(out=onesb_sb, in_=onesb)
            nc.sync.dma_start(out=exp4_sb, in_=exp4)
            nc.sync.dma_start(out=bias_sb, in_=bias_bc)

            for b in range(BL):
                for i in range(nt):
                    row0 = b * nq + i * TQ
                    qt = qt_pool.tile([D, TQ], bf16)
                    nc.sync.dma_start_transpose(qt, q[row0 : row0 + TQ, :])

                    st = st_pool.tile([KB, TQ], f32)
                    nc.tensor.matmul(st, kblk_sb[:, b, :], qt, start=True, stop=True)
                    et = et_pool.tile([KB, TQ], bf16)
                    nc.scalar.activation(
                        et, st, mybir.ActivationFunctionType.Exp,
                        bias=mb_sb[:, b : b + 1], scale=1.0,
                    )
                    r4 = r4_pool.tile([H, TQ], f32)
                    nc.tensor.matmul(r4, onesb_sb, et, start=True, stop=True)
                    rinv = rinv_pool.tile([H, TQ], f32)
                    nc.vector.reciprocal(rinv, r4)
                    rinvb = rinv_pool.tile([H, TQ], bf16)
                    nc.vector.tensor_copy(rinvb, rinv)
                    rx = rx_pool.tile([KB, TQ], f32)
                    nc.tensor.matmul(rx, exp4_sb, rinvb, start=True, stop=True)
                    en = en_pool.tile([KB, TQ], bf16)
                    nc.vector.tensor_mul(en, et, rx)

                    u = u_pool.tile([128, ntile, D], f32)
                    for t in range(ntile):
                        nc.tensor.matmul(
                            u[:, t, :],
                            en[:, t * 128 : (t + 1) * 128],
                            vw_sb[:, b, :],
                            start=True,
                            stop=True,
                        )
                    osb = o_pool.tile([128, ntile, D], f32)
                    nc.vector.tensor_add(
                        osb.rearrange("p t d -> p (t d)"),
                        u.rearrange("p t d -> p (t d)"),
                        bias_sb,
                    )
                    nc.sync.dma_start(
                        out=o[row0 : row0 + TQ].rearrange("(t p) d -> p t d", p=128),
                        in_=osb,
                    )
    nc.compile()
    return nc


def _get_program(mode: str):
    if mode not in _PROG_CACHE:
        _PROG_CACHE[mode] = _build_bf16() if mode == "bf16" else _build(mode)
    return _PROG_CACHE[mode]


def _host_prep(Q, K, V, mask, W_out, b_out, mode=None):
    mode = mode or DEFAULT_MODE
    if mode == "bf16":
        import ml_dtypes

        bf = ml_dtypes.bfloat16
        Q = np.asarray(Q, dtype=np.float32)
        K = np.asarray(K, dtype=np.float32)
        V = np.asarray(V, dtype=np.float32)
        W_out = np.asarray(W_out, dtype=np.float32)
        b_out = np.asarray(b_out, dtype=np.float32)
        mask = np.asarray(mask)

        Kblk = np.zeros((B, D, KB), np.float32)
        VW = np.zeros((B, KB, D), np.float32)
        mb = np.zeros((B, KB, 1), np.float32)
        for h in range(H):
            ds, ks = h * DH, h * NK
            Kblk[:, ds : ds + DH, ks : ks + NK] = (
                K[:, :, ds : ds + DH].transpose(0, 2, 1) / SCALE
            )
            VW[:, ks : ks + NK, :] = V[:, :, ds : ds + DH] @ W_out[:, ds : ds + DH].T
            mb[:, ks : ks + NK, 0] = np.where(mask, 0.0, MASK_NEG)
        onesb = np.zeros((KB, H), np.float32)
        for h in range(H):
            onesb[h * NK : (h + 1) * NK, h] = 1.0
        bias_bc = np.tile(b_out[None, :], (128, TQ // 128)).astype(np.float32)
        Qb = Q.astype(bf)
        Kblkb = Kblk.astype(bf)
        VWb = VW.astype(bf)
        onesbb = onesb.astype(bf)
        exp4b = onesb.T.copy().astype(bf)

        in_maps = []
        for c in range(NCORES):
            sl = slice(c * BL, (c + 1) * BL)
            in_maps.append(
                {
                    "q": Qb[sl].reshape(BL * NQ, D),
                    "kblk": Kblkb[sl],
                    "vw": VWb[sl],
                    "mbias": mb[sl],
                    "onesb": onesbb,
                    "exp4": exp4b,
                    "bias_bc": bias_bc,
                }
            )
        return in_maps

    Q = np.ascontiguousarray(np.asarray(Q, dtype=np.float32))
    K = np.asarray(K, dtype=np.float32)
    V = np.asarray(V, dtype=np.float32)
    W_out = np.asarray(W_out, dtype=np.float32)
    b_out = np.asarray(b_out, dtype=np.float32)
    mask = np.asarray(mask)

    Kblk = np.zeros((B, D, KB), np.float32)
    VW = np.zeros((B, KB, D), np.float32)
    mb = np.zeros((B, KB, 1), np.float32)
    for h in range(H):
        ds, ks = h * DH, h * NK
        Kblk[:, ds : ds + DH, ks : ks + NK] = (
            K[:, :, ds : ds + DH].transpose(0, 2, 1) / SCALE
        )
        VW[:, ks : ks + NK, :] = V[:, :, ds : ds + DH] @ W_out[:, ds : ds + DH].T
        mb[:, ks : ks + NK, 0] = np.where(mask, 0.0, MASK_NEG)
    M124 = np.zeros((KB, KB), np.float32)
    for h in range(H):
        M124[h * NK : (h + 1) * NK, h * NK : (h + 1) * NK] = 1.0
    bias_bc = np.tile(b_out[None, :], (128, TQ // 128)).astype(np.float32)

    in_maps = []
    for c in range(NCORES):
        sl = slice(c * BL, (c + 1) * BL)
        in_maps.append(
            {
                "q": Q[sl].reshape(BL * NQ, D),
                "kblk": Kblk[sl],
                "vw": VW[sl],
                "mbias": mb[sl],
                "m124": M124,
                "bias_bc": bias_bc,
            }
        )
    return in_maps


def _run(in_maps, mode: str, **kwargs):
    nc = _get_program(mode)
    return run_bass_kernel_spmd(nc, in_maps, list(range(NCORES)), **kwargs)


DEFAULT_MODE = os.environ.get("ATTN_MM_MODE", "f32r")


def kernel(Q, K, V, mask, W_out, b_out):
    in_maps = _host_prep(Q, K, V, mask, W_out, b_out)
    res = _run(in_maps, DEFAULT_MODE)
    out = np.empty((B, NQ, D), np.float32)
    for c in range(NCORES):
        out[c * BL : (c + 1) * BL] = res.results[c]["o"].reshape(BL, NQ, D)
    return out


# revision 9
# speedup vs baseline: 14.3433x; 14.3433x over previous
"""Multi-head cross attention (B=32, Nq=16384, Nk=31, d_model=64, H=4) on 8 trn2 cores.

Strategy: pure data parallel over batch (4 batches per core). Per batch b the
whole attention is restructured so the only large tensor (Q) is streamed once:

  scores^T[k,q] = (Kblk/sqrt(dh)).T @ Q^T      Kblk: [64, 124] block-diag per head
  E = exp(scores^T + mask_bias)                mask_bias: 0 / -1e30 per k-row
  R = M124.T @ E                               M124: [124,124] block-ones -> per-head row sums
  En = E / R                                   softmax weights (transposed layout)
  out[q,:] = En.T @ VW + b_out                 VW[31h:31h+31,:] = V_h @ W_out[:,hblk].T

Q^T tiles are produced on-chip with PE transposes of naturally-loaded Q tiles.
"""

import os
import sys

for _p in ("/opt/trn_rl_repo", "/opt/pypackages",
           "/root/.axon_site/_ro/trn_rl_repo", "/root/.axon_site/_ro/pypackages"):
    if os.path.isdir(_p) and _p not in sys.path:
        sys.path.insert(0, _p)

import math
import numpy as np

import concourse.bass as bass
import concourse.tile as tile
from concourse import bacc, mybir
from concourse.bass_utils import run_bass_kernel_spmd
from concourse.masks import make_identity

B, NQ, NK, D = 32, 16384, 31, 64
H, DH = 4, 16
SCALE = math.sqrt(DH)
NCORES = 8
BL = B // NCORES          # batches per core
TQ = 512                  # queries per inner iteration
NT = NQ // TQ             # iterations per batch
KB = H * NK               # 124 stacked key rows

MASK_NEG = -1e30

_PROG_CACHE: dict = {}


def _build(mode: str = "f32r", nq: int = NQ):
    """Build the per-core Bass program. mode: 'f32' | 'f32r' for matmul inputs."""
    f32 = mybir.dt.float32
    mmdt = mybir.dt.float32r if mode == "f32r" else f32

    def mmcast(ap):
        return ap

    nt = nq // TQ
    nc = bacc.Bacc("TRN2", target_bir_lowering=False, debug=False, num_devices=NCORES)
    q = nc.dram_tensor("q", [BL * nq, D], f32, kind="ExternalInput").ap()
    kblk = nc.dram_tensor("kblk", [BL, D, KB], mmdt, kind="ExternalInput").ap()
    vw = nc.dram_tensor("vw", [BL, KB, D], mmdt, kind="ExternalInput").ap()
    mbias = nc.dram_tensor("mbias", [BL, KB, 1], f32, kind="ExternalInput").ap()
    m124 = nc.dram_tensor("m124", [KB, KB], mmdt, kind="ExternalInput").ap()
    bias_bc = nc.dram_tensor("bias_bc", [128, (TQ // 128) * D], f32,
                             kind="ExternalInput").ap()
    o = nc.dram_tensor("o", [BL * nq, D], f32, kind="ExternalOutput").ap()

    ntile = TQ // 128  # q-subtiles of 128 per iteration

    with tile.TileContext(nc) as tc:
        with (
            tc.tile_pool(name="singles", bufs=1) as singles,
            tc.tile_pool(name="qin", bufs=3) as qin_pool,
            tc.tile_pool(name="qt_ps", bufs=2, space="PSUM") as qtps_pool,
            tc.tile_pool(name="qt_sb", bufs=2) as qtsb_pool,
            tc.tile_pool(name="st", bufs=2, space="PSUM") as st_pool,
            tc.tile_pool(name="et", bufs=2) as et_pool,
            tc.tile_pool(name="r", bufs=2, space="PSUM") as r_pool,
            tc.tile_pool(name="rinv", bufs=2) as rinv_pool,
            tc.tile_pool(name="en", bufs=2) as en_pool,
            tc.tile_pool(name="u", bufs=2, space="PSUM") as u_pool,
            tc.tile_pool(name="osb", bufs=3) as o_pool,
        ):
            ident = singles.tile([128, 128], f32)
            make_identity(nc, ident)
            kblk_sb = singles.tile([D, BL, KB], mmdt)
            vw_sb = singles.tile([KB, BL, D], mmdt)
            mb_sb = singles.tile([KB, BL], f32)
            m124_sb = singles.tile([KB, KB], mmdt)
            bias_sb = singles.tile([128, ntile * D], f32)
            for b in range(BL):
                nc.sync.dma_start(out=kblk_sb[:, b, :], in_=kblk[b])
                nc.sync.dma_start(out=vw_sb[:, b, :], in_=vw[b])
                nc.sync.dma_start(out=mb_sb[:, b : b + 1], in_=mbias[b])
            nc.sync.dma_start(out=m124_sb, in_=m124)
            nc.sync.dma_start(out=bias_sb, in_=bias_bc)

            for b in range(BL):
                for i in range(nt):
                    row0 = b * nq + i * TQ
                    qin = qin_pool.tile([128, ntile, D], f32)
                    nc.sync.dma_start(
                        out=qin,
                        in_=q[row0 : row0 + TQ].rearrange("(t p) d -> p t d", p=128),
                    )
                    qt_ps = qtps_pool.tile([D, ntile, 128], f32)
                    for t in range(ntile):
                        nc.tensor.transpose(qt_ps[:, t, :], qin[:, t, :], ident)
                    qt_sb = qtsb_pool.tile([D, ntile, 128], mmdt)
                    nc.scalar.copy(qt_sb, qt_ps)

                    st = st_pool.tile([KB, TQ], f32)
                    nc.tensor.matmul(
                        st,
                        mmcast(kblk_sb[:, b, :]),
                        mmcast(qt_sb.rearrange("d t p -> d (t p)")),
                        start=True,
                        stop=True,
                    )
                    et = et_pool.tile([KB, TQ], mmdt)
                    nc.scalar.activation(
                        et, st, mybir.ActivationFunctionType.Exp,
                        bias=mb_sb[:, b : b + 1], scale=1.0,
                    )
                    r = r_pool.tile([KB, TQ], f32)
                    nc.tensor.matmul(r, mmcast(m124_sb), mmcast(et), start=True, stop=True)
                    rinv = rinv_pool.tile([KB, TQ], f32)
                    nc.vector.reciprocal(rinv, r)
                    en = en_pool.tile([KB, TQ], mmdt)
                    nc.vector.tensor_mul(en, et, rinv)

                    u = u_pool.tile([128, ntile, D], f32)
                    for t in range(ntile):
                        nc.tensor.matmul(
                            u[:, t, :],
                            mmcast(en[:, t * 128 : (t + 1) * 128]),
                            mmcast(vw_sb[:, b, :]),
                            start=True,
                            stop=True,
                        )
                    osb = o_pool.tile([128, ntile, D], f32)
                    nc.vector.tensor_add(
                        osb.rearrange("p t d -> p (t d)"),
                        u.rearrange("p t d -> p (t d)"),
                        bias_sb,
                    )
                    nc.sync.dma_start(
                        out=o[row0 : row0 + TQ].rearrange("(t p) d -> p t d", p=128),
                        in_=osb,
                    )
    nc.compile()
    return nc


def _build_bf16(nq: int = NQ):
    """bf16 path. Q arrives host-pre-transposed as qT [BL, 64, nq] so every DMA
    is wide and natural; the output is produced transposed (oT [BL, 64, nq]) and
    un-transposed on the host. Softmax normalization: per-head row-sum matmul ->
    reciprocal_approx_fast -> PE broadcast-expand -> multiply."""
    f32 = mybir.dt.float32
    bf16 = mybir.dt.bfloat16
    nt = nq // TQ

    nc = bacc.Bacc("TRN2", target_bir_lowering=False, debug=False, num_devices=NCORES)
    qT = nc.dram_tensor("qT", [BL, D, nq], bf16, kind="ExternalInput").ap()
    kblk = nc.dram_tensor("kblk", [BL, D, KB], bf16, kind="ExternalInput").ap()
    vw = nc.dram_tensor("vw", [BL, KB, D], bf16, kind="ExternalInput").ap()
    mbias = nc.dram_tensor("mbias", [BL, KB, 1], f32, kind="ExternalInput").ap()
    onesb = nc.dram_tensor("onesb", [KB, H], bf16, kind="ExternalInput").ap()
    exp4 = nc.dram_tensor("exp4", [H, KB], bf16, kind="ExternalInput").ap()
    biasT = nc.dram_tensor("biasT", [D, 1], f32, kind="ExternalInput").ap()
    oT = nc.dram_tensor("oT", [BL, D, nq], f32, kind="ExternalOutput").ap()

    with tile.TileContext(nc) as tc:
        with (
            tc.tile_pool(name="singles", bufs=1) as singles,
            tc.tile_pool(name="qt", bufs=3) as qt_pool,
            tc.tile_pool(name="st", bufs=2, space="PSUM") as st_pool,
            tc.tile_pool(name="et", bufs=2) as et_pool,
            tc.tile_pool(name="r4", bufs=2, space="PSUM") as r4_pool,
            tc.tile_pool(name="rinv", bufs=2) as rinv_pool,
            tc.tile_pool(name="rx", bufs=2, space="PSUM") as rx_pool,
            tc.tile_pool(name="en", bufs=2) as en_pool,
            tc.tile_pool(name="u", bufs=2, space="PSUM") as u_pool,
            tc.tile_pool(name="osb", bufs=3) as o_pool,
        ):
            kblk_sb = singles.tile([D, BL, KB], bf16)
            vw_sb = singles.tile([KB, BL, D], bf16)
            mb_sb = singles.tile([KB, BL], f32)
            onesb_sb = singles.tile([KB, H], bf16)
            exp4_sb = singles.tile([H, KB], bf16)
            biasT_sb = singles.tile([D, 1], f32)
            for b in range(BL):
                nc.sync.dma_start(out=kblk_sb[:, b, :], in_=kblk[b])
                nc.sync.dma_start(out=vw_sb[:, b, :], in_=vw[b])
                nc.sync.dma_start(out=mb_sb[:, b : b + 1], in_=mbias[b])
            nc.sync.dma_start(out=onesb_sb, in_=onesb)
            nc.sync.dma_start(out=exp4_sb, in_=exp4)
            nc.sync.dma_start(out=biasT_sb, in_=biasT)

            for b in range(BL):
                for i in range(nt):
                    col0 = i * TQ
                    qt = qt_pool.tile([D, TQ], bf16)
                    nc.sync.dma_start(out=qt, in_=qT[b, :, col0 : col0 + TQ])

                    st = st_pool.tile([KB, TQ], f32)
                    nc.tensor.matmul(st, kblk_sb[:, b, :], qt, start=True, stop=True)
                    et = et_pool.tile([KB, TQ], bf16)
                    nc.scalar.activation(
                        et, st, mybir.ActivationFunctionType.Exp,
                        bias=mb_sb[:, b : b + 1], scale=1.0,
                    )
                    r4 = r4_pool.tile([H, TQ], f32)
                    nc.tensor.matmul(r4, onesb_sb, et, start=True, stop=True)
                    rinv = rinv_pool.tile([H, TQ], f32)
                    nc.vector.reciprocal_approx_fast(rinv, r4)
                    rinvb = rinv_pool.tile([H, TQ], bf16)
                    nc.vector.tensor_copy(rinvb, rinv)
                    rx = rx_pool.tile([KB, TQ], f32)
                    nc.tensor.matmul(rx, exp4_sb, rinvb, start=True, stop=True)
                    en = en_pool.tile([KB, TQ], bf16)
                    nc.vector.tensor_mul(en, et, rx)

                    u = u_pool.tile([D, TQ], f32)
                    nc.tensor.matmul(u, vw_sb[:, b, :], en, start=True, stop=True)
                    osb = o_pool.tile([D, TQ], f32)
                    nc.vector.tensor_scalar_add(osb, u, biasT_sb)
                    nc.sync.dma_start(out=oT[b, :, col0 : col0 + TQ], in_=osb)
    nc.compile()
    return nc


def _get_program(mode: str):
    if mode not in _PROG_CACHE:
        _PROG_CACHE[mode] = _build_bf16() if mode == "bf16" else _build(mode)
    return _PROG_CACHE[mode]


def _host_prep(Q, K, V, mask, W_out, b_out, mode=None):
    mode = mode or DEFAULT_MODE
    if mode == "bf16":
        import ml_dtypes

        bf = ml_dtypes.bfloat16
        Q = np.asarray(Q, dtype=np.float32)
        K = np.asarray(K, dtype=np.float32)
        V = np.asarray(V, dtype=np.float32)
        W_out = np.asarray(W_out, dtype=np.float32)
        b_out = np.asarray(b_out, dtype=np.float32)
        mask = np.asarray(mask)

        Kblk = np.zeros((B, D, KB), np.float32)
        VW = np.zeros((B, KB, D), np.float32)
        mb = np.zeros((B, KB, 1), np.float32)
        for h in range(H):
            ds, ks = h * DH, h * NK
            Kblk[:, ds : ds + DH, ks : ks + NK] = (
                K[:, :, ds : ds + DH].transpose(0, 2, 1) / SCALE
            )
            VW[:, ks : ks + NK, :] = V[:, :, ds : ds + DH] @ W_out[:, ds : ds + DH].T
            mb[:, ks : ks + NK, 0] = np.where(mask, 0.0, MASK_NEG)
        onesb = np.zeros((KB, H), np.float32)
        for h in range(H):
            onesb[h * NK : (h + 1) * NK, h] = 1.0
        QTb = np.ascontiguousarray(Q.transpose(0, 2, 1)).astype(bf)
        Kblkb = Kblk.astype(bf)
        VWb = VW.astype(bf)
        onesbb = onesb.astype(bf)
        exp4b = onesb.T.copy().astype(bf)
        biasT = b_out[:, None].astype(np.float32)

        in_maps = []
        for c in range(NCORES):
            sl = slice(c * BL, (c + 1) * BL)
            in_maps.append(
                {
                    "qT": QTb[sl],
                    "kblk": Kblkb[sl],
                    "vw": VWb[sl],
                    "mbias": mb[sl],
                    "onesb": onesbb,
                    "exp4": exp4b,
                    "biasT": biasT,
                }
            )
        return in_maps

    Q = np.ascontiguousarray(np.asarray(Q, dtype=np.float32))
    K = np.asarray(K, dtype=np.float32)
    V = np.asarray(V, dtype=np.float32)
    W_out = np.asarray(W_out, dtype=np.float32)
    b_out = np.asarray(b_out, dtype=np.float32)
    mask = np.asarray(mask)

    Kblk = np.zeros((B, D, KB), np.float32)
    VW = np.zeros((B, KB, D), np.float32)
    mb = np.zeros((B, KB, 1), np.float32)
    for h in range(H):
        ds, ks = h * DH, h * NK
        Kblk[:, ds : ds + DH, ks : ks + NK] = (
            K[:, :, ds : ds + DH].transpose(0, 2, 1) / SCALE
        )
        VW[:, ks : ks + NK, :] = V[:, :, ds : ds + DH] @ W_out[:, ds : ds + DH].T
        mb[:, ks : ks + NK, 0] = np.where(mask, 0.0, MASK_NEG)
    M124 = np.zeros((KB, KB), np.float32)
    for h in range(H):
        M124[h * NK : (h + 1) * NK, h * NK : (h + 1) * NK] = 1.0
    bias_bc = np.tile(b_out[None, :], (128, TQ // 128)).astype(np.float32)

    in_maps = []
    for c in range(NCORES):
        sl = slice(c * BL, (c + 1) * BL)
        in_maps.append(
            {
                "q": Q[sl].reshape(BL * NQ, D),
                "kblk": Kblk[sl],
                "vw": VW[sl],
                "mbias": mb[sl],
                "m124": M124,
                "bias_bc": bias_bc,
            }
        )
    return in_maps


def _run(in_maps, mode: str, **kwargs):
    nc = _get_program(mode)
    return run_bass_kernel_spmd(nc, in_maps, list(range(NCORES)), **kwargs)


DEFAULT_MODE = os.environ.get("ATTN_MM_MODE", "f32r")


def kernel(Q, K, V, mask, W_out, b_out):
    in_maps = _host_prep(Q, K, V, mask, W_out, b_out, DEFAULT_MODE)
    res = _run(in_maps, DEFAULT_MODE)
    out = np.empty((B, NQ, D), np.float32)
    for c in range(NCORES):
        if DEFAULT_MODE == "bf16":
            out[c * BL : (c + 1) * BL] = res.results[c]["oT"].transpose(0, 2, 1)
        else:
            out[c * BL : (c + 1) * BL] = res.results[c]["o"].reshape(BL, NQ, D)
    return out
